# revision 2
# baseline (speedup 1.0000x reference)
"""Trainium2 Bass kernel for the windowed bidirectional LSTM encoder.

Semantics (derived from the reference): each direction is a plain LSTM cell
chain over a token stream of length 2S-1 (windows overlap, so tokens repeat:
fwd stream = x0,x1,x1,x2,x2,...,x511; bwd stream = x1,x0,x2,x1,...,x511).
The output is the per-feature running max over all 2S-1 hidden states of each
direction, concatenated: emb = [max_t h_f(t) | max_t h_b(t)] -> (B, 2H).

Distribution (v3): sequence-parallel as v2 (16 segments/direction, stride 64,
W=8 warmup; every core runs 4 chains of L=72 steps, full B=64 per chain,
slots [f,f,b,b]) — but the compute is restructured around PE/ACT economy:

* Chain-pair matmuls: the two same-direction chains share weights, so every
  wih/whh matmul covers both chains at once (N=128 instead of 2x N=64).
  Halves the PE instruction count.
* Token-repeat reuse: each token's u = bias + wih@x is computed ONCE into an
  open PSUM accumulation group; the repeat step re-accumulates W@dh on top of
  the previous z (z' = z + W@(h_new - h_old)), never recomputing u. Fwd
  repeats at lag 1 (1 bank-pair), bwd at lag 3 (3 rotating bank-pairs);
  2 + 6 = 8 PSUM banks exactly. stop= is only set by the last matmul per
  bank of the group-closing step (sim zero-region = 1 bank).
* Bank/gate split: each pair's z lives in 2 adjacent banks as
  [g g i i | f f o o] x (chain, batch), so the two ACT reads per pair-step
  are flat contiguous PSUM ranges (tanh N=256, sigmoid N=768) and every
  pointwise op is pair-wide (256 cols) with contiguous APs.
* tanh(c) is one joint ACT instruction over both pairs (512 cols).

Per pair-step: PE z = bias(K=4 indicator) + wih@x (fresh only) + whh@{h|dh};
ACT tanh/sigmoid; DVE m1=tg*si, v=sf*c, h=so*th, running max, dh; Pool c=m1+v.
"""

import numpy as np
import ml_dtypes

import concourse.bass as bass
import concourse.mybir as mybir
from concourse import bacc
from concourse.tile import TileContext
from concourse.bass_utils import run_bass_kernel_spmd

F32 = mybir.dt.float32
BF16 = mybir.dt.bfloat16
AF = mybir.ActivationFunctionType
ALU = mybir.AluOpType

S = 512
B = 64
E = 256
H = 256
NCORES = 8
KT = 2                    # k-tiles (contraction 256 = 2x128)
GT = 8                    # gate tiles (4H = 1024 = 8x128)

NSEG = 16                 # segments per direction
STRIDE = 64               # even stream stride between segment starts
W = 8                     # warmup steps
L = STRIDE + W            # steps per chain = 72
NPAIR = 2                 # chain pairs per core: [fwd, bwd]
NT = 2 * S - 1            # real stream length = 1023

# gate-tile order [g g | i i | f f | o o]; orig (PyTorch) blocks i:0,1 f:2,3
# g:4,5 o:6,7
GATE_ROW_PERM = [4, 5, 0, 1, 2, 3, 6, 7]


def _rt_fwd(t):
    return (t + 1) // 2


def _rt_bwd(t):
    return t // 2 + 1 if t % 2 == 0 else (t - 1) // 2


RT = [_rt_fwd, _rt_bwd]


def _sched(d, t):
    """Per-direction step schedule.

    Returns (fresh_u, rhs, closes) where rhs in (None, 'h', 'dh') and
    closes=True when this step's matmuls end the PSUM accumulation group.
    fwd: tokens repeat at lag 1 (pair = odd t, even t+1).
    bwd: tokens repeat at lag 3 (fresh = even t, reuse = t+3).
    """
    if d == 0:
        if t == 0:
            return True, None, True
        if t % 2 == 1:
            return True, 'h', t == L - 1
        return False, 'dh', True
    else:
        if t == 0:
            return True, None, L <= 3
        if t == 1:
            return True, 'h', True
        if t % 2 == 0:
            return True, 'h', t + 3 > L - 1
        # odd t >= 3; at t == 3 the lag-3 predecessor h is zero -> plain h
        return False, ('h' if t == 3 else 'dh'), True


# wblob (bf16): [ wih: 2*KT*GT*128 | biasmat: 2*2*128 | ind4: 512
#                 | whh: 2*KT*GT*128 ]
# xblob (bf16): per STEP, t-major: [t, k, pair, c, b] -> 512 cols/step;
# only fresh-u steps are populated.
TE = 4                    # steps covered by the early X DMA
XROW = 512
WIH_OFF = 0
BM_OFF = WIH_OFF + 2 * KT * GT * 128
IND_OFF = BM_OFF + 2 * 2 * 128
WHH_OFF = IND_OFF + 512
WCOLS = WHH_OFF + 2 * KT * GT * 128
XCOLS = L * XROW


def _build_program():
    nc = bacc.Bacc(None, target_bir_lowering=False)
    wblob = nc.dram_tensor("wblob", [128, WCOLS], BF16, kind="ExternalInput")
    xblob = nc.dram_tensor("xblob", [128, XCOLS], BF16, kind="ExternalInput")
    out = nc.dram_tensor("out", [128, NPAIR * 3 * 256], BF16,
                         kind="ExternalOutput")

    with TileContext(nc) as tc:
        with (
            tc.tile_pool(name="const", bufs=1) as const_pool,
            tc.tile_pool(name="work", bufs=3) as work,
            tc.tile_pool(name="acc", bufs=1) as acc,
            tc.tile_pool(name="zp", bufs=1, space="PSUM") as zpool,
        ):
            wearly_sb = const_pool.tile([128, WHH_OFF], BF16)
            nc.sync.dma_start(wearly_sb[:], wblob[:, 0:WHH_OFF])
            xearly_sb = const_pool.tile([128, TE * XROW], BF16)
            nc.sync.dma_start(xearly_sb[:], xblob[:, 0:TE * XROW])
            whh_sb = const_pool.tile([128, 2 * KT * GT * 128], BF16)
            nc.sync.dma_start(whh_sb[:], wblob[:, WHH_OFF:WCOLS])
            xrest_sb = const_pool.tile([128, (L - TE) * XROW], BF16)
            nc.sync.dma_start(xrest_sb[:], xblob[:, TE * XROW:])

            def x_ap(t, k, pair):
                # [128, 128] fresh-u rhs for both chains of `pair`
                off = k * 256 + pair * 128
                if t < TE:
                    return xearly_sb[:, t * XROW + off:t * XROW + off + 128]
                o = (t - TE) * XROW + off
                return xrest_sb[:, o:o + 128]

            def wih_ap(d, k, t8):
                off = WIH_OFF + ((d * KT + k) * GT + t8) * 128
                return wearly_sb[:, off:off + 128]

            def whh_ap(d, k, t8):
                off = ((d * KT + k) * GT + t8) * 128
                return whh_sb[:, off:off + 128]

            def biasmat_ap(d, bank):
                off = BM_OFF + (d * 2 + bank) * 128
                return wearly_sb[0:4, off:off + 128]

            ind4 = wearly_sb[0:4, IND_OFF:IND_OFF + 512]

            # PSUM: fwd pair = 1 two-bank tile; bwd pair = 3 rotating tiles
            zf = zpool.tile([128, 1024], F32, tag="zf", name="zf")
            zb = [
                zpool.tile([128, 1024], F32, tag=f"zb{r}", name=f"zb{r}")
                for r in range(3)
            ]

            # per-epoch max accumulators (pair-wide): e0 = warmup [0,W),
            # e1 = body [W, L-1), e2 = final step
            hmax = [
                [
                    acc.tile([128, 256], BF16, tag=f"hmax{p}_{e}",
                             name=f"hmax{p}_{e}")
                    for e in range(3)
                ]
                for p in range(NPAIR)
            ]
            for p in range(NPAIR):
                for e in range(3):
                    nc.gpsimd.memset(hmax[p][e][:], -3.0e9)

            h_ring = [[None] * 4 for _ in range(NPAIR)]
            c_prev = [None]  # joint [128, 512] tile of previous step

            def step_mm(pair, t, dh_tile):
                d = pair
                fresh, rhs, closes = _sched(d, t)
                z = zf if pair == 0 else zb[t % 3]
                if fresh:
                    nc.tensor.matmul(z[:, 0:512], biasmat_ap(d, 0), ind4,
                                     start=True, stop=False)
                    nc.tensor.matmul(z[:, 512:1024], biasmat_ap(d, 1), ind4,
                                     start=True, stop=False)
                    for t8 in range(GT):
                        zs = z[:, t8 * 128:(t8 + 1) * 128]
                        for k in range(KT):
                            last = rhs is None and k == KT - 1
                            nc.tensor.matmul(
                                zs, wih_ap(d, k, t8), x_ap(t, k, pair),
                                start=False,
                                stop=closes and last and t8 in (3, 7),
                            )
                if rhs is not None:
                    if rhs == 'h':
                        hr = h_ring[pair][(t - 1) % 4]
                    else:
                        hr = dh_tile
                    for t8 in range(GT):
                        zs = z[:, t8 * 128:(t8 + 1) * 128]
                        for k in range(KT):
                            nc.tensor.matmul(
                                zs, whh_ap(d, k, t8),
                                hr[:, k * 128:(k + 1) * 128],
                                start=False,
                                stop=closes and k == KT - 1 and t8 in (3, 7),
                            )

            dh = [None, None]
            for t in range(L):
                salls = []
                cjoint = work.tile([128, 512], BF16, tag="cj",
                                   name=f"cj_{t}")
                for pair in range(NPAIR):
                    step_mm(pair, t, dh[pair])
                    z = zf if pair == 0 else zb[t % 3]
                    sall = work.tile([128, 1024], BF16, tag=f"sall{pair}",
                                     name=f"sall{pair}_{t}")
                    nc.scalar.activation(sall[:, 0:256], z[:, 0:256], AF.Tanh)
                    nc.scalar.activation(sall[:, 256:1024], z[:, 256:1024],
                                         AF.Sigmoid)
                    salls.append(sall)
                    tg = sall[:, 0:256]
                    si = sall[:, 256:512]
                    sf = sall[:, 512:768]
                    cslice = cjoint[:, pair * 256:(pair + 1) * 256]
                    if t == 0:
                        nc.vector.tensor_tensor(cslice, tg, si, ALU.mult)
                    else:
                        m1 = work.tile([128, 256], BF16, tag=f"m1{pair}",
                                       name=f"m1{pair}_{t}")
                        nc.vector.tensor_tensor(m1[:], tg, si, ALU.mult)
                        v = work.tile([128, 256], BF16, tag=f"v{pair}",
                                      name=f"v{pair}_{t}")
                        nc.vector.tensor_tensor(
                            v[:], sf, c_prev[0][:, pair * 256:(pair + 1) * 256],
                            ALU.mult)
                        nc.gpsimd.tensor_tensor(cslice, m1[:], v[:], ALU.add)
                c_prev[0] = cjoint
                th = work.tile([128, 512], BF16, tag="th", name=f"th_{t}")
                nc.scalar.activation(th[:], cjoint[:], AF.Tanh)
                e = 0 if t < W else (1 if t < L - 1 else 2)
                for pair in range(NPAIR):
                    so = salls[pair][:, 768:1024]
                    h = work.tile([128, 256], BF16, tag=f"h{pair}_{t % 4}",
                                  bufs=1, name=f"h{pair}_{t}")
                    nc.vector.tensor_tensor(
                        h[:], so, th[:, pair * 256:(pair + 1) * 256], ALU.mult)
                    h_ring[pair][t % 4] = h
                    nc.vector.tensor_tensor(
                        hmax[pair][e][:], hmax[pair][e][:], h[:], ALU.max)
                # dh for next step's reuse matmuls
                if t % 2 == 1 and t + 1 <= L - 1:
                    d0 = work.tile([128, 256], BF16, tag="dh0", bufs=2,
                                   name=f"dh0_{t}")
                    nc.vector.tensor_tensor(
                        d0[:], h_ring[0][t % 4][:],
                        h_ring[0][(t - 1) % 4][:], ALU.subtract)
                    dh[0] = d0
                if t % 2 == 0 and t >= 4 and t + 1 <= L - 1:
                    d1 = work.tile([128, 256], BF16, tag="dh1", bufs=2,
                                   name=f"dh1_{t}")
                    nc.vector.tensor_tensor(
                        d1[:], h_ring[1][t % 4][:],
                        h_ring[1][(t - 3) % 4][:], ALU.subtract)
                    dh[1] = d1

            for p in range(NPAIR):
                for e in range(3):
                    off = (p * 3 + e) * 256
                    nc.sync.dma_start(out[:, off:off + 256], hmax[p][e][:])

    nc.compile()
    return nc


def _chain_meta():
    """Global chain table: (dir, seg_idx, aw) per (core, slot).

    slot = pair*2 + c; chains of a pair are segments 2*core and 2*core+1.
    """
    meta = []
    for core in range(NCORES):
        row = []
        for slot in range(2 * NPAIR):
            d = slot // 2
            j = 2 * core + (slot % 2)
            aw = 0 if j == 0 else STRIDE * j - W
            row.append((d, j, aw))
        meta.append(row)
    return meta


def _pack_blobs(X, weights):
    bf = ml_dtypes.bfloat16
    perm = np.concatenate(
        [np.arange(r * 128, (r + 1) * 128) for r in GATE_ROW_PERM]
    )

    def lhsT_img(Wm):
        img = np.empty((128, KT * GT * 128), np.float32)
        for k in range(KT):
            for t8 in range(GT):
                blockT = Wm[t8 * 128:(t8 + 1) * 128, k * 128:(k + 1) * 128].T
                img[:, (k * GT + t8) * 128:(k * GT + t8 + 1) * 128] = blockT
        return img

    wimg = np.zeros((128, WCOLS), np.float32)
    for d, nm in enumerate("fb"):
        wih_p = weights[f"wih_{nm}"][perm].copy()
        whh_p = weights[f"whh_{nm}"][perm].copy()
        bias_p = (weights[f"bih_{nm}"] + weights[f"bhh_{nm}"])[perm].copy()
        wimg[:, WIH_OFF + d * 2048:WIH_OFF + (d + 1) * 2048] = lhsT_img(wih_p)
        wimg[:, WHH_OFF + d * 2048:WHH_OFF + (d + 1) * 2048] = lhsT_img(whh_p)
        for bank in range(2):
            off = BM_OFF + (d * 2 + bank) * 128
            for j in range(4):
                wimg[j, off:off + 128] = bias_p[(bank * 4 + j) * 128:
                                                (bank * 4 + j + 1) * 128]
    for j in range(4):
        wimg[j, IND_OFF + j * 128:IND_OFF + (j + 1) * 128] = 1.0
    wimg = wimg.astype(bf)

    # X as [k, 128, tok, b]
    Xt = np.ascontiguousarray(
        np.transpose(X.reshape(S, B, KT, 128), (2, 3, 0, 1))
    )

    meta = _chain_meta()
    xblobs = []
    for core in range(NCORES):
        img = np.zeros((128, XCOLS), np.float32)
        for slot in range(2 * NPAIR):
            d, j, aw = meta[core][slot]
            pair, c = slot // 2, slot % 2
            lo = aw // 2
            for t in range(L):
                fresh, _, _ = _sched(d, t)
                if not fresh:
                    continue
                gid = min(lo + RT[d](t), S - 1)
                for k in range(KT):
                    col = t * XROW + k * 256 + pair * 128 + c * 64
                    img[:, col:col + B] = Xt[k][:, gid, :]
        xblobs.append(img.astype(bf))
    return wimg, xblobs


_PROGRAM_CACHE = {}


def _get_program():
    if "nc" not in _PROGRAM_CACHE:
        _PROGRAM_CACHE["nc"] = _build_program()
    return _PROGRAM_CACHE["nc"]


def _run(inputs, trace=False):
    X = np.asarray(inputs["inputs"], np.float32)
    wimg, xblobs = _pack_blobs(X, inputs)
    nc = _get_program()
    in_maps = [{"wblob": wimg, "xblob": xb} for xb in xblobs]
    res = run_bass_kernel_spmd(
        nc, in_maps, core_ids=list(range(NCORES)), trace=trace
    )
    meta = _chain_meta()
    emb = np.full((2, B, H), -np.inf, np.float32)
    for core in range(NCORES):
        o = np.asarray(res.results[core]["out"], np.float32)
        for slot in range(2 * NPAIR):
            d, j, aw = meta[core][slot]
            pair, c = slot // 2, slot % 2
            epochs = [1]
            if j == 0:
                epochs.append(0)
            if aw + L - 1 < NT:
                epochs.append(2)
            for e in epochs:
                off = (pair * 3 + e) * 256
                blk = o[:, off:off + 256].reshape(128, 2, 2, 64)
                # feature X*128+p of chain c lives at [p, X, c, b]
                cur = blk[:, :, c, :]              # (p, X, b)
                cur = np.transpose(cur, (2, 1, 0)).reshape(B, H)
                emb[d] = np.maximum(emb[d], cur)
    return np.concatenate([emb[0], emb[1]], axis=-1), res


def kernel(**inputs):
    emb, _ = _run(inputs, trace=False)
    return emb


# revision 5
# speedup vs baseline: 1.1605x; 1.1605x over previous
"""Trainium2 Bass kernel for the windowed bidirectional LSTM encoder.

Semantics (derived from the reference): each direction is a plain LSTM cell
chain over a token stream of length 2S-1 (windows overlap, so tokens repeat:
fwd stream = x0,x1,x1,x2,x2,...,x511; bwd stream = x1,x0,x2,x1,...,x511).
The output is the per-feature running max over all 2S-1 hidden states of each
direction, concatenated: emb = [max_t h_f(t) | max_t h_b(t)] -> (B, 2H).

Distribution (v3): sequence-parallel as v2 (16 segments/direction, stride 64,
W=8 warmup; every core runs 4 chains of L=72 steps, full B=64 per chain,
slots [f,f,b,b]) — but the compute is restructured around PE/ACT economy:

* Chain-pair matmuls: the two same-direction chains share weights, so every
  wih/whh matmul covers both chains at once (N=128 instead of 2x N=64).
  Halves the PE instruction count.
* Token-repeat reuse: each token's u = bias + wih@x is computed ONCE into an
  open PSUM accumulation group; the repeat step re-accumulates W@dh on top of
  the previous z (z' = z + W@(h_new - h_old)), never recomputing u. Fwd
  repeats at lag 1 (1 bank-pair), bwd at lag 3 (3 rotating bank-pairs);
  2 + 6 = 8 PSUM banks exactly. stop= is only set by the last matmul per
  bank of the group-closing step (sim zero-region = 1 bank).
* Bank/gate split: each pair's z lives in 2 adjacent banks as
  [g g i i | f f o o] x (chain, batch), so the two ACT reads per pair-step
  are flat contiguous PSUM ranges (tanh N=256, sigmoid N=768) and every
  pointwise op is pair-wide (256 cols) with contiguous APs.
* tanh(c) is one joint ACT instruction over both pairs (512 cols).

Per pair-step: PE z = bias(K=4 indicator) + wih@x (fresh only) + whh@{h|dh};
ACT tanh/sigmoid; DVE m1=tg*si, v=sf*c, h=so*th, running max, dh; Pool c=m1+v.
"""

import numpy as np
import ml_dtypes

import concourse.bass as bass
import concourse.mybir as mybir
from concourse import bacc
from concourse.tile import TileContext
from concourse.bass_utils import run_bass_kernel_spmd

F32 = mybir.dt.float32
BF16 = mybir.dt.bfloat16
AF = mybir.ActivationFunctionType
ALU = mybir.AluOpType

S = 512
B = 64
E = 256
H = 256
NCORES = 8
KT = 2                    # k-tiles (contraction 256 = 2x128)
GT = 8                    # gate tiles (4H = 1024 = 8x128)

NSEG = 16                 # segments per direction
STRIDE = 64               # even stream stride between segment starts
W = 8                     # warmup steps
L = STRIDE + W            # steps per chain = 72
NPAIR = 2                 # chain pairs per core: [fwd, bwd]
NT = 2 * S - 1            # real stream length = 1023

# gate-tile order [g g | i i | f f | o o]; orig (PyTorch) blocks i:0,1 f:2,3
# g:4,5 o:6,7
GATE_ROW_PERM = [4, 5, 0, 1, 2, 3, 6, 7]


def _rt_fwd(t):
    return (t + 1) // 2


def _rt_bwd(t):
    return t // 2 + 1 if t % 2 == 0 else (t - 1) // 2


RT = [_rt_fwd, _rt_bwd]


def _sched(d, t):
    """Per-direction step schedule.

    Returns (fresh_u, rhs, closes) where rhs in (None, 'h', 'dh') and
    closes=True when this step's matmuls end the PSUM accumulation group.
    fwd: tokens repeat at lag 1 (pair = odd t, even t+1).
    bwd: tokens repeat at lag 3 (fresh = even t, reuse = t+3).
    """
    if d == 0:
        if t == 0:
            return True, None, True
        if t % 2 == 1:
            return True, 'h', t == L - 1
        return False, 'dh', True
    else:
        if t == 0:
            return True, None, L <= 3
        if t == 1:
            return True, 'h', True
        if t % 2 == 0:
            return True, 'h', t + 3 > L - 1
        # odd t >= 3; at t == 3 the lag-3 predecessor h is zero -> plain h
        return False, ('h' if t == 3 else 'dh'), True


# wblob (bf16): [ wih: 2*KT*GT*128 | biasmat: 2*2*128 | ind4: 512
#                 | whh: 2*KT*GT*128 ]
# xblob (bf16): per STEP, t-major: [t, k, pair, c, b] -> 512 cols/step;
# only fresh-u steps are populated.
TE = 4                    # steps covered by the early X DMA
XROW = 512
WIH_OFF = 0
BM_OFF = WIH_OFF + 2 * KT * GT * 128
IND_OFF = BM_OFF + 2 * 2 * 128
WHH_OFF = IND_OFF + 512
WCOLS = WHH_OFF + 2 * KT * GT * 128
XCOLS = L * XROW


def _build_program():
    nc = bacc.Bacc(None, target_bir_lowering=False)
    wblob = nc.dram_tensor("wblob", [128, WCOLS], BF16, kind="ExternalInput")
    xblob = nc.dram_tensor("xblob", [128, XCOLS], BF16, kind="ExternalInput")
    out = nc.dram_tensor("out", [128, NPAIR * 3 * 256], BF16,
                         kind="ExternalOutput")

    with TileContext(nc) as tc:
        with (
            tc.tile_pool(name="const", bufs=1) as const_pool,
            tc.tile_pool(name="work", bufs=3) as work,
            tc.tile_pool(name="acc", bufs=1) as acc,
            tc.tile_pool(name="zp", bufs=1, space="PSUM") as zpool,
        ):
            wearly_sb = const_pool.tile([128, WHH_OFF], BF16)
            nc.sync.dma_start(wearly_sb[:], wblob[:, 0:WHH_OFF])
            xearly_sb = const_pool.tile([128, TE * XROW], BF16)
            nc.sync.dma_start(xearly_sb[:], xblob[:, 0:TE * XROW])
            whh_sb = const_pool.tile([128, 2 * KT * GT * 128], BF16)
            nc.sync.dma_start(whh_sb[:], wblob[:, WHH_OFF:WCOLS])
            xrest_sb = const_pool.tile([128, (L - TE) * XROW], BF16)
            nc.sync.dma_start(xrest_sb[:], xblob[:, TE * XROW:])

            def x_ap(t, k, pair):
                # [128, 128] fresh-u rhs for both chains of `pair`
                off = k * 256 + pair * 128
                if t < TE:
                    return xearly_sb[:, t * XROW + off:t * XROW + off + 128]
                o = (t - TE) * XROW + off
                return xrest_sb[:, o:o + 128]

            def wih_ap(d, k, t8):
                off = WIH_OFF + ((d * KT + k) * GT + t8) * 128
                return wearly_sb[:, off:off + 128]

            def whh_ap(d, k, t8):
                off = ((d * KT + k) * GT + t8) * 128
                return whh_sb[:, off:off + 128]

            def biasmat_ap(d, bank):
                off = BM_OFF + (d * 2 + bank) * 128
                return wearly_sb[0:4, off:off + 128]

            ind4 = wearly_sb[0:4, IND_OFF:IND_OFF + 512]

            # PSUM: fwd pair = 1 two-bank tile; bwd pair = 3 rotating tiles
            zf = zpool.tile([128, 1024], F32, tag="zf", name="zf")
            zb = [
                zpool.tile([128, 1024], F32, tag=f"zb{r}", name=f"zb{r}")
                for r in range(3)
            ]

            # per-epoch max accumulators (pair-wide): e0 = warmup [0,W),
            # e1 = body [W, L-1), e2 = final step
            hmax = [
                [
                    acc.tile([128, 256], BF16, tag=f"hmax{p}_{e}",
                             name=f"hmax{p}_{e}")
                    for e in range(3)
                ]
                for p in range(NPAIR)
            ]
            for p in range(NPAIR):
                for e in range(3):
                    nc.gpsimd.memset(hmax[p][e][:], -3.0e9)

            h_ring = [[None] * 4 for _ in range(NPAIR)]

            def step_mm(pair, t, dh_tile):
                d = pair
                fresh, rhs, closes = _sched(d, t)
                z = zf if pair == 0 else zb[t % 3]
                if fresh:
                    nc.tensor.matmul(z[:, 0:512], biasmat_ap(d, 0), ind4,
                                     start=True, stop=False)
                    nc.tensor.matmul(z[:, 512:1024], biasmat_ap(d, 1), ind4,
                                     start=True, stop=False)
                    for t8 in range(GT):
                        zs = z[:, t8 * 128:(t8 + 1) * 128]
                        for k in range(KT):
                            last = rhs is None and k == KT - 1
                            nc.tensor.matmul(
                                zs, wih_ap(d, k, t8), x_ap(t, k, pair),
                                start=False,
                                stop=closes and last and t8 in (3, 7),
                            )
                if rhs is not None:
                    if rhs == 'h':
                        hr = h_ring[pair][(t - 1) % 4]
                    else:
                        hr = dh_tile
                    for t8 in range(GT):
                        zs = z[:, t8 * 128:(t8 + 1) * 128]
                        for k in range(KT):
                            nc.tensor.matmul(
                                zs, whh_ap(d, k, t8),
                                hr[:, k * 128:(k + 1) * 128],
                                start=False,
                                stop=closes and k == KT - 1 and t8 in (3, 7),
                            )

            c_prev = [None, None]
            dh = [None, None]
            for t in range(L):
                e = 0 if t < W else (1 if t < L - 1 else 2)
                for pair in range(NPAIR):
                    step_mm(pair, t, dh[pair])
                    z = zf if pair == 0 else zb[t % 3]
                    sall = work.tile([128, 1024], BF16, tag=f"sall{pair}",
                                     name=f"sall{pair}_{t}")
                    nc.scalar.activation(sall[:, 0:256], z[:, 0:256], AF.Tanh)
                    nc.scalar.activation(sall[:, 256:1024], z[:, 256:1024],
                                         AF.Sigmoid)
                    tg = sall[:, 0:256]
                    si = sall[:, 256:512]
                    sf = sall[:, 512:768]
                    so = sall[:, 768:1024]
                    cnew = work.tile([128, 256], BF16, tag=f"c{pair}",
                                     name=f"c{pair}_{t}")
                    if t == 0:
                        nc.vector.tensor_tensor(cnew[:], tg, si, ALU.mult)
                    else:
                        m1 = work.tile([128, 256], BF16, tag=f"m1{pair}",
                                       name=f"m1{pair}_{t}")
                        nc.gpsimd.tensor_tensor(m1[:], tg, si, ALU.mult)
                        v = work.tile([128, 256], BF16, tag=f"v{pair}",
                                      name=f"v{pair}_{t}")
                        nc.vector.tensor_tensor(
                            v[:], sf, c_prev[pair][:], ALU.mult)
                        nc.vector.tensor_tensor(cnew[:], m1[:], v[:], ALU.add)
                    c_prev[pair] = cnew
                    th = work.tile([128, 256], BF16, tag=f"th{pair}",
                                   name=f"th{pair}_{t}")
                    nc.scalar.activation(th[:], cnew[:], AF.Tanh)
                    h = work.tile([128, 256], BF16, tag=f"h{pair}_{t % 4}",
                                  bufs=1, name=f"h{pair}_{t}")
                    nc.vector.tensor_tensor(h[:], so, th[:], ALU.mult)
                    h_ring[pair][t % 4] = h
                    nc.vector.tensor_tensor(
                        hmax[pair][e][:], hmax[pair][e][:], h[:], ALU.max)
                    # dh for the next reuse step of this direction
                    if pair == 0 and t % 2 == 1 and t + 1 <= L - 1:
                        d0 = work.tile([128, 256], BF16, tag="dh0", bufs=2,
                                       name=f"dh0_{t}")
                        nc.vector.tensor_tensor(
                            d0[:], h[:], h_ring[0][(t - 1) % 4][:],
                            ALU.subtract)
                        dh[0] = d0
                    if pair == 1 and t % 2 == 0 and t >= 4 and t + 1 <= L - 1:
                        d1 = work.tile([128, 256], BF16, tag="dh1", bufs=2,
                                       name=f"dh1_{t}")
                        nc.vector.tensor_tensor(
                            d1[:], h[:], h_ring[1][(t - 3) % 4][:],
                            ALU.subtract)
                        dh[1] = d1

            for p in range(NPAIR):
                for e in range(3):
                    off = (p * 3 + e) * 256
                    nc.sync.dma_start(out[:, off:off + 256], hmax[p][e][:])

    nc.compile()
    return nc


def _chain_meta():
    """Global chain table: (dir, seg_idx, aw) per (core, slot).

    slot = pair*2 + c; chains of a pair are segments 2*core and 2*core+1.
    """
    meta = []
    for core in range(NCORES):
        row = []
        for slot in range(2 * NPAIR):
            d = slot // 2
            j = 2 * core + (slot % 2)
            aw = 0 if j == 0 else STRIDE * j - W
            row.append((d, j, aw))
        meta.append(row)
    return meta


def _pack_blobs(X, weights):
    bf = ml_dtypes.bfloat16
    perm = np.concatenate(
        [np.arange(r * 128, (r + 1) * 128) for r in GATE_ROW_PERM]
    )

    def lhsT_img(Wm):
        img = np.empty((128, KT * GT * 128), np.float32)
        for k in range(KT):
            for t8 in range(GT):
                blockT = Wm[t8 * 128:(t8 + 1) * 128, k * 128:(k + 1) * 128].T
                img[:, (k * GT + t8) * 128:(k * GT + t8 + 1) * 128] = blockT
        return img

    wimg = np.zeros((128, WCOLS), np.float32)
    for d, nm in enumerate("fb"):
        wih_p = weights[f"wih_{nm}"][perm].copy()
        whh_p = weights[f"whh_{nm}"][perm].copy()
        bias_p = (weights[f"bih_{nm}"] + weights[f"bhh_{nm}"])[perm].copy()
        wimg[:, WIH_OFF + d * 2048:WIH_OFF + (d + 1) * 2048] = lhsT_img(wih_p)
        wimg[:, WHH_OFF + d * 2048:WHH_OFF + (d + 1) * 2048] = lhsT_img(whh_p)
        for bank in range(2):
            off = BM_OFF + (d * 2 + bank) * 128
            for j in range(4):
                wimg[j, off:off + 128] = bias_p[(bank * 4 + j) * 128:
                                                (bank * 4 + j + 1) * 128]
    for j in range(4):
        wimg[j, IND_OFF + j * 128:IND_OFF + (j + 1) * 128] = 1.0
    wimg = wimg.astype(bf)

    # X as [k, 128, tok, b]
    Xt = np.ascontiguousarray(
        np.transpose(X.reshape(S, B, KT, 128), (2, 3, 0, 1))
    )

    meta = _chain_meta()
    xblobs = []
    for core in range(NCORES):
        img = np.zeros((128, XCOLS), np.float32)
        for slot in range(2 * NPAIR):
            d, j, aw = meta[core][slot]
            pair, c = slot // 2, slot % 2
            lo = aw // 2
            for t in range(L):
                fresh, _, _ = _sched(d, t)
                if not fresh:
                    continue
                gid = min(lo + RT[d](t), S - 1)
                for k in range(KT):
                    col = t * XROW + k * 256 + pair * 128 + c * 64
                    img[:, col:col + B] = Xt[k][:, gid, :]
        xblobs.append(img.astype(bf))
    return wimg, xblobs


_PROGRAM_CACHE = {}


def _get_program():
    if "nc" not in _PROGRAM_CACHE:
        _PROGRAM_CACHE["nc"] = _build_program()
    return _PROGRAM_CACHE["nc"]


def _run(inputs, trace=False):
    X = np.asarray(inputs["inputs"], np.float32)
    wimg, xblobs = _pack_blobs(X, inputs)
    nc = _get_program()
    in_maps = [{"wblob": wimg, "xblob": xb} for xb in xblobs]
    res = run_bass_kernel_spmd(
        nc, in_maps, core_ids=list(range(NCORES)), trace=trace
    )
    meta = _chain_meta()
    emb = np.full((2, B, H), -np.inf, np.float32)
    for core in range(NCORES):
        o = np.asarray(res.results[core]["out"], np.float32)
        for slot in range(2 * NPAIR):
            d, j, aw = meta[core][slot]
            pair, c = slot // 2, slot % 2
            epochs = [1]
            if j == 0:
                epochs.append(0)
            if aw + L - 1 < NT:
                epochs.append(2)
            for e in epochs:
                off = (pair * 3 + e) * 256
                blk = o[:, off:off + 256].reshape(128, 2, 2, 64)
                # feature X*128+p of chain c lives at [p, X, c, b]
                cur = blk[:, :, c, :]              # (p, X, b)
                cur = np.transpose(cur, (2, 1, 0)).reshape(B, H)
                emb[d] = np.maximum(emb[d], cur)
    return np.concatenate([emb[0], emb[1]], axis=-1), res


def kernel(**inputs):
    emb, _ = _run(inputs, trace=False)
    return emb


# revision 8
# speedup vs baseline: 1.3828x; 1.1916x over previous
"""Trainium2 Bass kernel for the windowed bidirectional LSTM encoder.

Semantics (derived from the reference): each direction is a plain LSTM cell
chain over a token stream of length 2S-1 (windows overlap, so tokens repeat:
fwd stream = x0,x1,x1,x2,x2,...,x511; bwd stream = x1,x0,x2,x1,...,x511).
The output is the per-feature running max over all 2S-1 hidden states of each
direction, concatenated: emb = [max_t h_f(t) | max_t h_b(t)] -> (B, 2H).

Distribution (v5): sequence-parallel as v2 (16 segments/direction, stride 64,
W=8 warmup; every core runs 4 chains of L=72 steps, full B=64 per chain,
slots [f,f,b,b]) with a delta-telescoping PE structure:

* Chain-pair matmuls: the two same-direction chains share weights, so every
  wih/whh matmul covers both chains at once (N=128 instead of 2x N=64).
* Eternal PSUM groups: each direction owns 2 alternating two-bank PSUM
  tiles whose accumulation group is opened once (bias + wih@x at t<2) and
  never restarted. Every later step re-accumulates only deltas:
      Z_t = Z_prev + wih@(x_gid(t) - x_gid(prev)) + whh@(h_{t-1} - h_prev-1)
  The bias is never re-added (it cancels), and the fwd direction's repeated
  tokens make its even-step wih delta exactly zero (skip all wih matmuls).
  Host packs the x blob as the per-step CLAMPED token deltas in fp32, cast
  to bf16. stop= fires only on each bank's last matmul (t >= L-2).
* Bank/gate split: each pair's z lives in 2 adjacent banks as
  [g g i i | f f o o] x (chain, batch), so ACT reads are flat contiguous
  PSUM ranges (tanh N=256, sigmoid N=768); all pointwise ops are pair-wide.

Per pair-step: PE z += wih@dx (fresh) + whh@dh; ACT tanh/sigmoid (PSUM);
Pool m1=tg*si (hides under the sigmoid); DVE v=sf*c, c=m1+v, h=so*th,
running per-epoch max, and the dh/dx-side h deltas; ACT th=tanh(c).
"""

import numpy as np
import ml_dtypes

import concourse.bass as bass
import concourse.mybir as mybir
from concourse import bacc
from concourse.tile import TileContext
from concourse.bass_utils import run_bass_kernel_spmd

F32 = mybir.dt.float32
BF16 = mybir.dt.bfloat16
AF = mybir.ActivationFunctionType
ALU = mybir.AluOpType

S = 512
B = 64
E = 256
H = 256
NCORES = 8
KT = 2                    # k-tiles (contraction 256 = 2x128)
GT = 8                    # gate tiles (4H = 1024 = 8x128)

NSEG = 16                 # segments per direction
STRIDE = 64               # even stream stride between segment starts
W = 8                     # warmup steps
L = STRIDE + W            # steps per chain = 72
NPAIR = 2                 # chain pairs per core: [fwd, bwd]
NT = 2 * S - 1            # real stream length = 1023

# gate-tile order [g g | i i | f f | o o]; orig (PyTorch) blocks i:0,1 f:2,3
# g:4,5 o:6,7
GATE_ROW_PERM = [4, 5, 0, 1, 2, 3, 6, 7]


def _rt_fwd(t):
    return (t + 1) // 2


def _rt_bwd(t):
    return t // 2 + 1 if t % 2 == 0 else (t - 1) // 2


RT = [_rt_fwd, _rt_bwd]


def _lag(d, t):
    """Steps since this step's PSUM bank was last written (0 = fresh start).

    fwd banks alternate per token-pair (lag 3 on pair hop, 1 inside a
    pair); bwd banks alternate per step (lag 2).
    """
    if d == 0:
        if t in (0, 1):
            return 0
        return 1 if t % 2 == 0 else 3
    else:
        return 0 if t in (0, 1) else 2


def _has_wih(d, t):
    """Does step t accumulate a nonzero wih@dx term?"""
    if d == 0:
        return t == 0 or t % 2 == 1   # even-step token repeats: dx == 0
    return True


def _bank_idx(d, t):
    if d == 0:
        return 0 if t == 0 else ((t + 1) // 2) % 2
    return t % 2


# wblob (bf16): [ wih: 2*KT*GT*128 | biasmat: 2*2*128 | ind4: 512
#                 | whh: 2*KT*GT*128 ]
# xblob (bf16): per STEP, t-major: [t, k, pair, c, b] -> 512 cols/step;
# holds token DELTAS (plain x on the two start steps), zero when unused.
TE = 4                    # steps covered by the early X DMA
XROW = 512
WIH_OFF = 0
BM_OFF = WIH_OFF + 2 * KT * GT * 128
IND_OFF = BM_OFF + 2 * 2 * 128
WHH_OFF = IND_OFF + 512
WCOLS = WHH_OFF + 2 * KT * GT * 128
XCOLS = L * XROW


def _build_program():
    nc = bacc.Bacc(None, target_bir_lowering=False)
    wblob = nc.dram_tensor("wblob", [128, WCOLS], BF16, kind="ExternalInput")
    xblob = nc.dram_tensor("xblob", [128, XCOLS], BF16, kind="ExternalInput")
    out = nc.dram_tensor("out", [128, NPAIR * 3 * 256], BF16,
                         kind="ExternalOutput")

    with TileContext(nc) as tc:
        with (
            tc.tile_pool(name="const", bufs=1) as const_pool,
            tc.tile_pool(name="work", bufs=3) as work,
            tc.tile_pool(name="acc", bufs=1) as acc,
            tc.tile_pool(name="zp", bufs=1, space="PSUM") as zpool,
        ):
            wearly_sb = const_pool.tile([128, WHH_OFF], BF16)
            nc.sync.dma_start(wearly_sb[:], wblob[:, 0:WHH_OFF])
            xearly_sb = const_pool.tile([128, TE * XROW], BF16)
            nc.sync.dma_start(xearly_sb[:], xblob[:, 0:TE * XROW])
            whh_sb = const_pool.tile([128, 2 * KT * GT * 128], BF16)
            nc.sync.dma_start(whh_sb[:], wblob[:, WHH_OFF:WCOLS])
            xrest_sb = const_pool.tile([128, (L - TE) * XROW], BF16)
            nc.sync.dma_start(xrest_sb[:], xblob[:, TE * XROW:])

            def x_ap(t, k, pair):
                # [128, 128] wih rhs (token delta) for both chains of `pair`
                off = k * 256 + pair * 128
                if t < TE:
                    return xearly_sb[:, t * XROW + off:t * XROW + off + 128]
                o = (t - TE) * XROW + off
                return xrest_sb[:, o:o + 128]

            def wih_ap(d, k, t8):
                off = WIH_OFF + ((d * KT + k) * GT + t8) * 128
                return wearly_sb[:, off:off + 128]

            def whh_ap(d, k, t8):
                off = ((d * KT + k) * GT + t8) * 128
                return whh_sb[:, off:off + 128]

            def biasmat_ap(d, bank):
                off = BM_OFF + (d * 2 + bank) * 128
                return wearly_sb[0:4, off:off + 128]

            ind4 = wearly_sb[0:4, IND_OFF:IND_OFF + 512]

            # PSUM: 2 alternating two-bank tiles per direction
            zt = [
                [
                    zpool.tile([128, 1024], F32, tag=f"z{d}{r}",
                               name=f"z{d}{r}")
                    for r in range(2)
                ]
                for d in range(2)
            ]

            # per-epoch max accumulators (pair-wide): e0 = warmup [0,W),
            # e1 = body [W, L-1), e2 = final step
            hmax = [
                [
                    acc.tile([128, 256], BF16, tag=f"hmax{p}_{e}",
                             name=f"hmax{p}_{e}")
                    for e in range(3)
                ]
                for p in range(NPAIR)
            ]
            for p in range(NPAIR):
                for e in range(3):
                    nc.gpsimd.memset(hmax[p][e][:], -3.0e9)

            h_ring = [[None] * 4 for _ in range(NPAIR)]

            def step_mm(pair, t, dh_tile):
                d = pair
                lag = _lag(d, t)
                closes = t >= L - 2
                z = zt[d][_bank_idx(d, t)]
                if lag == 0:
                    nc.tensor.matmul(z[:, 0:512], biasmat_ap(d, 0), ind4,
                                     start=True, stop=False)
                    nc.tensor.matmul(z[:, 512:1024], biasmat_ap(d, 1), ind4,
                                     start=True, stop=False)
                if _has_wih(d, t):
                    for t8 in range(GT):
                        zs = z[:, t8 * 128:(t8 + 1) * 128]
                        for k in range(KT):
                            last = t == 0 and k == KT - 1
                            nc.tensor.matmul(
                                zs, wih_ap(d, k, t8), x_ap(t, k, pair),
                                start=False,
                                stop=closes and last and t8 in (3, 7),
                            )
                if t > 0:
                    # rhs: plain h_{t-1} while the bank's old h-term is zero,
                    # else the prepared delta tile
                    plain = (t == 1 or (d == 0 and t == 3)
                             or (d == 1 and t == 2))
                    hr = h_ring[pair][(t - 1) % 4] if plain else dh_tile
                    for t8 in range(GT):
                        zs = z[:, t8 * 128:(t8 + 1) * 128]
                        for k in range(KT):
                            nc.tensor.matmul(
                                zs, whh_ap(d, k, t8),
                                hr[:, k * 128:(k + 1) * 128],
                                start=False,
                                stop=closes and k == KT - 1 and t8 in (3, 7),
                            )

            c_prev = [None, None]
            dh = [None, None]
            for t in range(L):
                e = 0 if t < W else (1 if t < L - 1 else 2)
                for pair in range(NPAIR):
                    step_mm(pair, t, dh[pair])
                    z = zt[pair][_bank_idx(pair, t)]
                    sall = work.tile([128, 1024], BF16, tag=f"sall{pair}",
                                     name=f"sall{pair}_{t}")
                    nc.scalar.activation(sall[:, 0:256], z[:, 0:256], AF.Tanh)
                    nc.scalar.activation(sall[:, 256:1024], z[:, 256:1024],
                                         AF.Sigmoid)
                    tg = sall[:, 0:256]
                    si = sall[:, 256:512]
                    sf = sall[:, 512:768]
                    so = sall[:, 768:1024]
                    cnew = work.tile([128, 256], BF16, tag=f"c{pair}",
                                     name=f"c{pair}_{t}")
                    if t == 0:
                        nc.vector.tensor_tensor(cnew[:], tg, si, ALU.mult)
                    else:
                        m1 = work.tile([128, 256], BF16, tag=f"m1{pair}",
                                       name=f"m1{pair}_{t}")
                        nc.gpsimd.tensor_tensor(m1[:], tg, si, ALU.mult)
                        v = work.tile([128, 256], BF16, tag=f"v{pair}",
                                      name=f"v{pair}_{t}")
                        nc.vector.tensor_tensor(
                            v[:], sf, c_prev[pair][:], ALU.mult)
                        nc.vector.tensor_tensor(cnew[:], m1[:], v[:], ALU.add)
                    c_prev[pair] = cnew
                    th = work.tile([128, 256], BF16, tag=f"th{pair}",
                                   name=f"th{pair}_{t}")
                    nc.scalar.activation(th[:], cnew[:], AF.Tanh)
                    h = work.tile([128, 256], BF16, tag=f"h{pair}_{t % 4}",
                                  bufs=1, name=f"h{pair}_{t}")
                    nc.vector.tensor_tensor(h[:], so, th[:], ALU.mult)
                    h_ring[pair][t % 4] = h
                    nc.vector.tensor_tensor(
                        hmax[pair][e][:], hmax[pair][e][:], h[:], ALU.max)
                    # delta rhs for this pair's next step
                    tn = t + 1
                    if tn <= L - 1:
                        if pair == 0:
                            lagh = 1 if tn % 2 == 0 else 3
                            need = tn >= 2 if tn % 2 == 0 else tn >= 5
                        else:
                            lagh = 2
                            need = tn >= 3
                        if need:
                            dnew = work.tile([128, 256], BF16,
                                             tag=f"dh{pair}", bufs=2,
                                             name=f"dh{pair}_{t}")
                            nc.vector.tensor_tensor(
                                dnew[:], h[:], h_ring[pair][(t - lagh) % 4][:],
                                ALU.subtract)
                            dh[pair] = dnew

            for p in range(NPAIR):
                for e in range(3):
                    off = (p * 3 + e) * 256
                    nc.sync.dma_start(out[:, off:off + 256], hmax[p][e][:])

    nc.compile()
    return nc


def _chain_meta():
    """Global chain table: (dir, seg_idx, aw) per (core, slot)."""
    meta = []
    for core in range(NCORES):
        row = []
        for slot in range(2 * NPAIR):
            d = slot // 2
            j = 2 * core + (slot % 2)
            aw = 0 if j == 0 else STRIDE * j - W
            row.append((d, j, aw))
        meta.append(row)
    return meta


def _pack_blobs(X, weights):
    bf = ml_dtypes.bfloat16
    perm = np.concatenate(
        [np.arange(r * 128, (r + 1) * 128) for r in GATE_ROW_PERM]
    )

    def lhsT_img(Wm):
        img = np.empty((128, KT * GT * 128), np.float32)
        for k in range(KT):
            for t8 in range(GT):
                blockT = Wm[t8 * 128:(t8 + 1) * 128, k * 128:(k + 1) * 128].T
                img[:, (k * GT + t8) * 128:(k * GT + t8 + 1) * 128] = blockT
        return img

    wimg = np.zeros((128, WCOLS), np.float32)
    for d, nm in enumerate("fb"):
        wih_p = weights[f"wih_{nm}"][perm].copy()
        whh_p = weights[f"whh_{nm}"][perm].copy()
        bias_p = (weights[f"bih_{nm}"] + weights[f"bhh_{nm}"])[perm].copy()
        wimg[:, WIH_OFF + d * 2048:WIH_OFF + (d + 1) * 2048] = lhsT_img(wih_p)
        wimg[:, WHH_OFF + d * 2048:WHH_OFF + (d + 1) * 2048] = lhsT_img(whh_p)
        for bank in range(2):
            off = BM_OFF + (d * 2 + bank) * 128
            for j in range(4):
                wimg[j, off:off + 128] = bias_p[(bank * 4 + j) * 128:
                                                (bank * 4 + j + 1) * 128]
    for j in range(4):
        wimg[j, IND_OFF + j * 128:IND_OFF + (j + 1) * 128] = 1.0
    wimg = wimg.astype(bf)

    # X as [k, 128, tok, b]
    Xt = np.ascontiguousarray(
        np.transpose(X.reshape(S, B, KT, 128), (2, 3, 0, 1))
    )

    meta = _chain_meta()
    xblobs = []
    for core in range(NCORES):
        img = np.zeros((128, XCOLS), np.float32)
        for slot in range(2 * NPAIR):
            d, j, aw = meta[core][slot]
            pair, c = slot // 2, slot % 2
            lo = aw // 2
            gid = [min(lo + RT[d](t), S - 1) for t in range(L)]
            # error-compensated deltas: track the bank's effective
            # accumulated token (fp32 sum of the emitted bf16 deltas) so
            # rounding error stays bounded instead of random-walking
            xeff = [None, None]
            for t in range(L):
                if not _has_wih(d, t):
                    continue
                bk = _bank_idx(d, t)
                tgt = Xt[:, :, gid[t], :]                    # (KT, 128, B)
                if xeff[bk] is None:
                    dxb = tgt.astype(bf).astype(np.float32)
                    xeff[bk] = dxb.copy()
                else:
                    dxb = (tgt - xeff[bk]).astype(bf).astype(np.float32)
                    xeff[bk] = xeff[bk] + dxb
                for k in range(KT):
                    col = t * XROW + k * 256 + pair * 128 + c * 64
                    img[:, col:col + B] = dxb[k]
        xblobs.append(img.astype(bf))
    return wimg, xblobs


_PROGRAM_CACHE = {}


def _get_program():
    if "nc" not in _PROGRAM_CACHE:
        _PROGRAM_CACHE["nc"] = _build_program()
    return _PROGRAM_CACHE["nc"]


def _run(inputs, trace=False):
    X = np.asarray(inputs["inputs"], np.float32)
    wimg, xblobs = _pack_blobs(X, inputs)
    nc = _get_program()
    in_maps = [{"wblob": wimg, "xblob": xb} for xb in xblobs]
    res = run_bass_kernel_spmd(
        nc, in_maps, core_ids=list(range(NCORES)), trace=trace
    )
    meta = _chain_meta()
    emb = np.full((2, B, H), -np.inf, np.float32)
    for core in range(NCORES):
        o = np.asarray(res.results[core]["out"], np.float32)
        for slot in range(2 * NPAIR):
            d, j, aw = meta[core][slot]
            pair, c = slot // 2, slot % 2
            epochs = [1]
            if j == 0:
                epochs.append(0)
            if aw + L - 1 < NT:
                epochs.append(2)
            for e in epochs:
                off = (pair * 3 + e) * 256
                blk = o[:, off:off + 256].reshape(128, 2, 2, 64)
                cur = blk[:, :, c, :]              # (p, X, b)
                cur = np.transpose(cur, (2, 1, 0)).reshape(B, H)
                emb[d] = np.maximum(emb[d], cur)
    return np.concatenate([emb[0], emb[1]], axis=-1), res


def kernel(**inputs):
    emb, _ = _run(inputs, trace=False)
    return emb


# revision 9
# speedup vs baseline: 1.6689x; 1.2069x over previous
"""Trainium2 Bass kernel for the windowed bidirectional LSTM encoder.

Semantics (derived from the reference): each direction is a plain LSTM cell
chain over a token stream of length 2S-1 (windows overlap, so tokens repeat:
fwd stream = x0,x1,x1,x2,x2,...,x511; bwd stream = x1,x0,x2,x1,...,x511).
The output is the per-feature running max over all 2S-1 hidden states of each
direction, concatenated: emb = [max_t h_f(t) | max_t h_b(t)] -> (B, 2H).

Distribution (v6): sequence-parallel, 32 segments per direction (stride 32,
W=8 warmup). Every core runs 8 chains of L=40 steps as 4 chain-PAIRS
(2 fwd pairs + 2 bwd pairs, full B=64 per chain); the pair is the compute
unit so matmuls cover both same-direction chains at once (N=128).

* Eternal PSUM groups with lag-1 delta telescoping: each pair owns one
  two-bank PSUM tile whose accumulation group is opened once (t<=1) and
  never restarted:
      Z_t = Z_{t-1} + wih@(x_g(t) - x_g(t-1)) + whh@(h_{t-1} - h_{t-2})
  Bias is added only at the start (it cancels in deltas); the fwd
  direction's repeated tokens make its even-step x-delta exactly zero, so
  fwd pairs skip wih matmuls on even steps. The host packs x deltas with
  fp32 error compensation (tracks each bank's effective accumulated token)
  so bf16 delta rounding stays bounded instead of random-walking.
* Bank/gate split: each pair's z = [g g i i | f f o o] x (chain, batch)
  across 2 adjacent banks -> ACT reads are flat PSUM ranges (tanh N=256,
  sigmoid N=768); all pointwise ops are pair-wide (256 cols) contiguous.
* 4 independent pair recurrences interleave on the engines, hiding the
  per-step dependency latency (PE z-matmuls -> ACT gates -> DVE/Pool cell
  update -> ACT tanh(c) -> DVE h -> PE next step).
"""

import numpy as np
import ml_dtypes

import concourse.bass as bass
import concourse.mybir as mybir
from concourse import bacc
from concourse.tile import TileContext
from concourse.bass_utils import run_bass_kernel_spmd

F32 = mybir.dt.float32
BF16 = mybir.dt.bfloat16
AF = mybir.ActivationFunctionType
ALU = mybir.AluOpType

S = 512
B = 64
E = 256
H = 256
NCORES = 8
KT = 2                    # k-tiles (contraction 256 = 2x128)
GT = 8                    # gate tiles (4H = 1024 = 8x128)

NSEG = 32                 # segments per direction
STRIDE = 32               # even stream stride between segment starts
W = 8                     # warmup steps
L = STRIDE + W            # steps per chain = 40
NPAIR = 4                 # chain pairs per core: [f, f, b, b]
NT = 2 * S - 1            # real stream length = 1023

# gate-tile order [g g | i i | f f | o o]; orig (PyTorch) blocks i:0,1 f:2,3
# g:4,5 o:6,7
GATE_ROW_PERM = [4, 5, 0, 1, 2, 3, 6, 7]


def _rt_fwd(t):
    return (t + 1) // 2


def _rt_bwd(t):
    return t // 2 + 1 if t % 2 == 0 else (t - 1) // 2


RT = [_rt_fwd, _rt_bwd]


def _pair_dir(p):
    return p // 2


def _has_wih(d, t):
    """Does step t accumulate a nonzero wih x-delta?"""
    if d == 0:
        return t == 0 or t % 2 == 1   # fwd even-step token repeats: dx == 0
    return True


# wblob (bf16): [ wih: 2*KT*GT*128 | biasmat: 2*2*128 | ind4: 512
#                 | whh: 2*KT*GT*128 ]
# xblob (bf16): per STEP, t-major: [t, k, pair, c, b] -> 1024 cols/step;
# holds error-compensated token DELTAS (plain x at t=0), zero when unused.
TE = 3                    # steps covered by the early X DMA
XROW = KT * NPAIR * 128   # 1024
WIH_OFF = 0
BM_OFF = WIH_OFF + 2 * KT * GT * 128
IND_OFF = BM_OFF + 2 * 2 * 128
WHH_OFF = IND_OFF + 512
WCOLS = WHH_OFF + 2 * KT * GT * 128
XCOLS = L * XROW


def _build_program():
    nc = bacc.Bacc(None, target_bir_lowering=False)
    wblob = nc.dram_tensor("wblob", [128, WCOLS], BF16, kind="ExternalInput")
    xblob = nc.dram_tensor("xblob", [128, XCOLS], BF16, kind="ExternalInput")
    out = nc.dram_tensor("out", [128, NPAIR * 3 * 256], BF16,
                         kind="ExternalOutput")

    with TileContext(nc) as tc:
        with (
            tc.tile_pool(name="const", bufs=1) as const_pool,
            tc.tile_pool(name="work", bufs=3) as work,
            tc.tile_pool(name="acc", bufs=1) as acc,
            tc.tile_pool(name="zp", bufs=1, space="PSUM") as zpool,
        ):
            wearly_sb = const_pool.tile([128, WHH_OFF], BF16)
            nc.sync.dma_start(wearly_sb[:], wblob[:, 0:WHH_OFF])
            xearly_sb = const_pool.tile([128, TE * XROW], BF16)
            nc.sync.dma_start(xearly_sb[:], xblob[:, 0:TE * XROW])
            whh_sb = const_pool.tile([128, 2 * KT * GT * 128], BF16)
            nc.sync.dma_start(whh_sb[:], wblob[:, WHH_OFF:WCOLS])
            xrest_sb = const_pool.tile([128, (L - TE) * XROW], BF16)
            nc.sync.dma_start(xrest_sb[:], xblob[:, TE * XROW:])

            def x_ap(t, k, pair):
                # [128, 128] wih rhs (token delta) for both chains of `pair`
                off = k * NPAIR * 128 + pair * 128
                if t < TE:
                    return xearly_sb[:, t * XROW + off:t * XROW + off + 128]
                o = (t - TE) * XROW + off
                return xrest_sb[:, o:o + 128]

            def wih_ap(d, k, t8):
                off = WIH_OFF + ((d * KT + k) * GT + t8) * 128
                return wearly_sb[:, off:off + 128]

            def whh_ap(d, k, t8):
                off = ((d * KT + k) * GT + t8) * 128
                return whh_sb[:, off:off + 128]

            def biasmat_ap(d, bank):
                off = BM_OFF + (d * 2 + bank) * 128
                return wearly_sb[0:4, off:off + 128]

            ind4 = wearly_sb[0:4, IND_OFF:IND_OFF + 512]

            # one eternal two-bank PSUM tile per pair
            zt = [
                zpool.tile([128, 1024], F32, tag=f"z{p}", name=f"z{p}")
                for p in range(NPAIR)
            ]

            # per-epoch max accumulators (pair-wide): e0 = warmup [0,W),
            # e1 = body [W, L-1), e2 = final step
            hmax = [
                [
                    acc.tile([128, 256], BF16, tag=f"hmax{p}_{e}",
                             name=f"hmax{p}_{e}")
                    for e in range(3)
                ]
                for p in range(NPAIR)
            ]
            for p in range(NPAIR):
                for e in range(3):
                    nc.gpsimd.memset(hmax[p][e][:], -3.0e9)

            h_prev = [[None, None] for _ in range(NPAIR)]  # h_{t-1}, h_{t-2}

            def step_mm(pair, t, dh_tile):
                d = _pair_dir(pair)
                closes = t == L - 1
                z = zt[pair]
                if t == 0:
                    nc.tensor.matmul(z[:, 0:512], biasmat_ap(d, 0), ind4,
                                     start=True, stop=False)
                    nc.tensor.matmul(z[:, 512:1024], biasmat_ap(d, 1), ind4,
                                     start=True, stop=False)
                if _has_wih(d, t):
                    for t8 in range(GT):
                        zs = z[:, t8 * 128:(t8 + 1) * 128]
                        for k in range(KT):
                            last = t == 0 and k == KT - 1
                            nc.tensor.matmul(
                                zs, wih_ap(d, k, t8), x_ap(t, k, pair),
                                start=False,
                                stop=closes and last and t8 in (3, 7),
                            )
                if t > 0:
                    hr = h_prev[pair][0] if t == 1 else dh_tile
                    for t8 in range(GT):
                        zs = z[:, t8 * 128:(t8 + 1) * 128]
                        for k in range(KT):
                            nc.tensor.matmul(
                                zs, whh_ap(d, k, t8),
                                hr[:, k * 128:(k + 1) * 128],
                                start=False,
                                stop=closes and k == KT - 1 and t8 in (3, 7),
                            )

            c_prev = [None] * NPAIR
            dh = [None] * NPAIR
            for t in range(L):
                e = 0 if t < W else (1 if t < L - 1 else 2)
                for pair in range(NPAIR):
                    step_mm(pair, t, dh[pair])
                    z = zt[pair]
                    sall = work.tile([128, 1024], BF16, tag=f"sall{pair}",
                                     name=f"sall{pair}_{t}")
                    nc.scalar.activation(sall[:, 0:256], z[:, 0:256], AF.Tanh)
                    nc.scalar.activation(sall[:, 256:1024], z[:, 256:1024],
                                         AF.Sigmoid)
                    tg = sall[:, 0:256]
                    si = sall[:, 256:512]
                    sf = sall[:, 512:768]
                    so = sall[:, 768:1024]
                    cnew = work.tile([128, 256], BF16, tag=f"c{pair}",
                                     name=f"c{pair}_{t}")
                    if t == 0:
                        nc.vector.tensor_tensor(cnew[:], tg, si, ALU.mult)
                    else:
                        m1 = work.tile([128, 256], BF16, tag=f"m1{pair}",
                                       name=f"m1{pair}_{t}")
                        nc.gpsimd.tensor_tensor(m1[:], tg, si, ALU.mult)
                        v = work.tile([128, 256], BF16, tag=f"v{pair}",
                                      name=f"v{pair}_{t}")
                        nc.vector.tensor_tensor(
                            v[:], sf, c_prev[pair][:], ALU.mult)
                        nc.vector.tensor_tensor(cnew[:], m1[:], v[:], ALU.add)
                    c_prev[pair] = cnew
                    th = work.tile([128, 256], BF16, tag=f"th{pair}",
                                   name=f"th{pair}_{t}")
                    nc.scalar.activation(th[:], cnew[:], AF.Tanh)
                    h = work.tile([128, 256], BF16, tag=f"h{pair}",
                                  name=f"h{pair}_{t}")
                    nc.vector.tensor_tensor(h[:], so, th[:], ALU.mult)
                    nc.vector.tensor_tensor(
                        hmax[pair][e][:], hmax[pair][e][:], h[:], ALU.max)
                    # lag-1 h delta for this pair's next step
                    if 1 <= t <= L - 2:
                        dnew = work.tile([128, 256], BF16, tag=f"dh{pair}",
                                         bufs=2, name=f"dh{pair}_{t}")
                        nc.vector.tensor_tensor(
                            dnew[:], h[:], h_prev[pair][0][:], ALU.subtract)
                        dh[pair] = dnew
                    h_prev[pair][1] = h_prev[pair][0]
                    h_prev[pair][0] = h

            for p in range(NPAIR):
                for e in range(3):
                    off = (p * 3 + e) * 256
                    nc.sync.dma_start(out[:, off:off + 256], hmax[p][e][:])

    nc.compile()
    return nc


def _chain_meta():
    """Global chain table: (dir, seg_idx, aw) per (core, slot).

    slot = pair*2 + c; fwd pairs 0,1 / bwd pairs 2,3; segment
    j = 4*core + (pair%2)*2 + c.
    """
    meta = []
    for core in range(NCORES):
        row = []
        for slot in range(2 * NPAIR):
            pair, c = slot // 2, slot % 2
            d = _pair_dir(pair)
            j = 4 * core + (pair % 2) * 2 + c
            aw = 0 if j == 0 else STRIDE * j - W
            row.append((d, j, aw))
        meta.append(row)
    return meta


def _pack_blobs(X, weights):
    bf = ml_dtypes.bfloat16
    perm = np.concatenate(
        [np.arange(r * 128, (r + 1) * 128) for r in GATE_ROW_PERM]
    )

    def lhsT_img(Wm):
        img = np.empty((128, KT * GT * 128), np.float32)
        for k in range(KT):
            for t8 in range(GT):
                blockT = Wm[t8 * 128:(t8 + 1) * 128, k * 128:(k + 1) * 128].T
                img[:, (k * GT + t8) * 128:(k * GT + t8 + 1) * 128] = blockT
        return img

    wimg = np.zeros((128, WCOLS), np.float32)
    for d, nm in enumerate("fb"):
        wih_p = weights[f"wih_{nm}"][perm].copy()
        whh_p = weights[f"whh_{nm}"][perm].copy()
        bias_p = (weights[f"bih_{nm}"] + weights[f"bhh_{nm}"])[perm].copy()
        wimg[:, WIH_OFF + d * 2048:WIH_OFF + (d + 1) * 2048] = lhsT_img(wih_p)
        wimg[:, WHH_OFF + d * 2048:WHH_OFF + (d + 1) * 2048] = lhsT_img(whh_p)
        for bank in range(2):
            off = BM_OFF + (d * 2 + bank) * 128
            for j in range(4):
                wimg[j, off:off + 128] = bias_p[(bank * 4 + j) * 128:
                                                (bank * 4 + j + 1) * 128]
    for j in range(4):
        wimg[j, IND_OFF + j * 128:IND_OFF + (j + 1) * 128] = 1.0
    wimg = wimg.astype(bf)

    # X as [k, 128, tok, b]
    Xt = np.ascontiguousarray(
        np.transpose(X.reshape(S, B, KT, 128), (2, 3, 0, 1))
    )

    meta = _chain_meta()
    xblobs = []
    for core in range(NCORES):
        img = np.zeros((128, XCOLS), np.float32)
        for slot in range(2 * NPAIR):
            d, j, aw = meta[core][slot]
            pair, c = slot // 2, slot % 2
            lo = aw // 2
            gid = [min(lo + RT[d](t), S - 1) for t in range(L)]
            # error-compensated deltas: track the bank's effective
            # accumulated token (fp32 sum of the emitted bf16 deltas)
            xeff = None
            for t in range(L):
                if not _has_wih(d, t):
                    continue
                tgt = Xt[:, :, gid[t], :]                    # (KT, 128, B)
                if xeff is None:
                    dxb = tgt.astype(bf).astype(np.float32)
                    xeff = dxb.copy()
                else:
                    dxb = (tgt - xeff).astype(bf).astype(np.float32)
                    xeff = xeff + dxb
                for k in range(KT):
                    col = t * XROW + k * NPAIR * 128 + pair * 128 + c * 64
                    img[:, col:col + B] = dxb[k]
        xblobs.append(img.astype(bf))
    return wimg, xblobs


_PROGRAM_CACHE = {}


def _get_program():
    if "nc" not in _PROGRAM_CACHE:
        _PROGRAM_CACHE["nc"] = _build_program()
    return _PROGRAM_CACHE["nc"]


def _run(inputs, trace=False):
    X = np.asarray(inputs["inputs"], np.float32)
    wimg, xblobs = _pack_blobs(X, inputs)
    nc = _get_program()
    in_maps = [{"wblob": wimg, "xblob": xb} for xb in xblobs]
    res = run_bass_kernel_spmd(
        nc, in_maps, core_ids=list(range(NCORES)), trace=trace
    )
    meta = _chain_meta()
    emb = np.full((2, B, H), -np.inf, np.float32)
    for core in range(NCORES):
        o = np.asarray(res.results[core]["out"], np.float32)
        for slot in range(2 * NPAIR):
            d, j, aw = meta[core][slot]
            pair, c = slot // 2, slot % 2
            epochs = [1]
            if j == 0:
                epochs.append(0)
            if aw + L - 1 < NT:
                epochs.append(2)
            for e in epochs:
                off = (pair * 3 + e) * 256
                blk = o[:, off:off + 256].reshape(128, 2, 2, 64)
                cur = blk[:, :, c, :]              # (p, X, b)
                cur = np.transpose(cur, (2, 1, 0)).reshape(B, H)
                emb[d] = np.maximum(emb[d], cur)
    return np.concatenate([emb[0], emb[1]], axis=-1), res


def kernel(**inputs):
    emb, _ = _run(inputs, trace=False)
    return emb


# revision 10
# speedup vs baseline: 1.9425x; 1.1640x over previous
"""Trainium2 Bass kernel for the windowed bidirectional LSTM encoder.

Semantics (derived from the reference): each direction is a plain LSTM cell
chain over a token stream of length 2S-1 (windows overlap, so tokens repeat:
fwd stream = x0,x1,x1,x2,x2,...,x511; bwd stream = x1,x0,x2,x1,...,x511).
The output is the per-feature running max over all 2S-1 hidden states of each
direction, concatenated: emb = [max_t h_f(t) | max_t h_b(t)] -> (B, 2H).

Distribution (v7): sequence-parallel, 32 segments per direction (stride 32,
W=8 warmup). Every core runs 8 chains of L=40 steps as 2 direction QUADS
(4 fwd chains + 4 bwd chains, full B=64 per chain). The quad is the compute
unit: all four same-direction chains share weights, so every wih/whh matmul
covers the quad (N=256), and the gate activations / cell updates are single
wide instructions.

* Eternal PSUM groups with lag-1 delta telescoping: each quad owns one
  four-bank PSUM tile ([g g | i i | f f | o o] x (chain, batch), one gate
  pair per bank) whose accumulation group is opened once (bias via a K=2
  indicator matmul at t=0) and never restarted:
      Z_t = Z_{t-1} + wih@(x_g(t) - x_g(t-1)) + whh@(h_{t-1} - h_{t-2})
  The fwd direction's repeated tokens make its even-step x-delta exactly
  zero (fwd skips wih on even steps). The host packs x deltas with fp32
  error compensation (tracks each bank's effective accumulated token) so
  bf16 delta rounding stays bounded instead of random-walking.
* ACT gate reads are flat contiguous PSUM ranges, split for latency:
  sigmoid(i,f) N=1024 first (releases v = f*c early), then tanh(g) N=512,
  then sigmoid(o) N=512; tanh(c) is one N=512 instruction per quad.
* The two quad recurrences run antiphase and interleave on the engines.
"""

import numpy as np
import ml_dtypes

import concourse.bass as bass
import concourse.mybir as mybir
from concourse import bacc
from concourse.tile import TileContext
from concourse.bass_utils import run_bass_kernel_spmd

F32 = mybir.dt.float32
BF16 = mybir.dt.bfloat16
AF = mybir.ActivationFunctionType
ALU = mybir.AluOpType

S = 512
B = 64
E = 256
H = 256
NCORES = 8
KT = 2                    # k-tiles (contraction 256 = 2x128)
GT = 8                    # gate tiles (4H = 1024 = 8x128)

NSEG = 32                 # segments per direction
STRIDE = 32               # even stream stride between segment starts
W = 8                     # warmup steps
L = STRIDE + W            # steps per chain = 40
NQ = 2                    # direction quads per core: [fwd, bwd]
CQ = 4                    # chains per quad
NT = 2 * S - 1            # real stream length = 1023

# gate-tile order [g g | i i | f f | o o]; orig (PyTorch) blocks i:0,1 f:2,3
# g:4,5 o:6,7
GATE_ROW_PERM = [4, 5, 0, 1, 2, 3, 6, 7]


def _rt_fwd(t):
    return (t + 1) // 2


def _rt_bwd(t):
    return t // 2 + 1 if t % 2 == 0 else (t - 1) // 2


RT = [_rt_fwd, _rt_bwd]


def _has_wih(d, t):
    """Does step t accumulate a nonzero wih x-delta?"""
    if d == 0:
        return t == 0 or t % 2 == 1   # fwd even-step token repeats: dx == 0
    return True


# wblob (bf16): [ wih: 2*KT*GT*128 | biasmat: 2*4*128 | ind2: 512
#                 | whh: 2*KT*GT*128 ]
# xblob (bf16): per STEP, t-major: [t, k, d, c4, b] -> 1024 cols/step;
# holds error-compensated token DELTAS (plain x at t=0), zero when unused.
TE = 3                    # steps covered by the early X DMA
XROW = KT * NQ * CQ * 64  # 1024
WIH_OFF = 0
BM_OFF = WIH_OFF + 2 * KT * GT * 128
IND_OFF = BM_OFF + 2 * 4 * 128
WHH_OFF = IND_OFF + 512
WCOLS = WHH_OFF + 2 * KT * GT * 128
XCOLS = L * XROW


def _build_program():
    nc = bacc.Bacc(None, target_bir_lowering=False)
    wblob = nc.dram_tensor("wblob", [128, WCOLS], BF16, kind="ExternalInput")
    xblob = nc.dram_tensor("xblob", [128, XCOLS], BF16, kind="ExternalInput")
    out = nc.dram_tensor("out", [128, NQ * 3 * 512], BF16,
                         kind="ExternalOutput")

    with TileContext(nc) as tc:
        with (
            tc.tile_pool(name="const", bufs=1) as const_pool,
            tc.tile_pool(name="work", bufs=3) as work,
            tc.tile_pool(name="acc", bufs=1) as acc,
            tc.tile_pool(name="zp", bufs=1, space="PSUM") as zpool,
        ):
            wearly_sb = const_pool.tile([128, WHH_OFF], BF16)
            nc.sync.dma_start(wearly_sb[:], wblob[:, 0:WHH_OFF])
            xearly_sb = const_pool.tile([128, TE * XROW], BF16)
            nc.sync.dma_start(xearly_sb[:], xblob[:, 0:TE * XROW])
            whh_sb = const_pool.tile([128, 2 * KT * GT * 128], BF16)
            nc.sync.dma_start(whh_sb[:], wblob[:, WHH_OFF:WCOLS])
            xrest_sb = const_pool.tile([128, (L - TE) * XROW], BF16)
            nc.sync.dma_start(xrest_sb[:], xblob[:, TE * XROW:])

            def x_ap(t, k, d):
                # [128, 256] wih rhs (token deltas) for all chains of quad d
                off = k * 512 + d * 256
                if t < TE:
                    return xearly_sb[:, t * XROW + off:t * XROW + off + 256]
                o = (t - TE) * XROW + off
                return xrest_sb[:, o:o + 256]

            def wih_ap(d, k, t8):
                off = WIH_OFF + ((d * KT + k) * GT + t8) * 128
                return wearly_sb[:, off:off + 128]

            def whh_ap(d, k, t8):
                off = ((d * KT + k) * GT + t8) * 128
                return whh_sb[:, off:off + 128]

            def biasmat_ap(d, bank):
                off = BM_OFF + (d * 4 + bank) * 128
                return wearly_sb[0:2, off:off + 128]

            ind2 = wearly_sb[0:2, IND_OFF:IND_OFF + 512]

            # one eternal four-bank PSUM tile per quad
            zt = [
                zpool.tile([128, 2048], F32, tag=f"z{d}", name=f"z{d}")
                for d in range(NQ)
            ]

            # per-epoch max accumulators (quad-wide): e0 = warmup [0,W),
            # e1 = body [W, L-1), e2 = final step
            hmax = [
                [
                    acc.tile([128, 512], BF16, tag=f"hmax{d}_{e}",
                             name=f"hmax{d}_{e}")
                    for e in range(3)
                ]
                for d in range(NQ)
            ]
            for d in range(NQ):
                for e in range(3):
                    nc.gpsimd.memset(hmax[d][e][:], -3.0e9)

            h_prev = [None] * NQ

            def step_mm(d, t, dh_tile):
                closes = t == L - 1
                z = zt[d]
                if t == 0:
                    for bank in range(4):
                        nc.tensor.matmul(
                            z[:, bank * 512:(bank + 1) * 512],
                            biasmat_ap(d, bank), ind2,
                            start=True, stop=False,
                        )
                if _has_wih(d, t):
                    for t8 in range(GT):
                        zs = z[:, t8 * 256:(t8 + 1) * 256]
                        for k in range(KT):
                            nc.tensor.matmul(
                                zs, wih_ap(d, k, t8), x_ap(t, k, d),
                                start=False, stop=False,
                            )
                if t > 0:
                    hr = h_prev[d] if t == 1 else dh_tile
                    for t8 in range(GT):
                        zs = z[:, t8 * 256:(t8 + 1) * 256]
                        for k in range(KT):
                            nc.tensor.matmul(
                                zs, whh_ap(d, k, t8),
                                hr[:, k * 256:(k + 1) * 256],
                                start=False,
                                stop=closes and k == KT - 1 and t8 % 2 == 1,
                            )

            c_prev = [None] * NQ
            dh = [None] * NQ
            for t in range(L):
                e = 0 if t < W else (1 if t < L - 1 else 2)
                for d in range(NQ):
                    step_mm(d, t, dh[d])
                    z = zt[d]
                    sall = work.tile([128, 2048], BF16, tag=f"sall{d}",
                                     name=f"sall{d}_{t}")
                    nc.scalar.activation(sall[:, 512:1536], z[:, 512:1536],
                                         AF.Sigmoid)
                    nc.scalar.activation(sall[:, 0:512], z[:, 0:512], AF.Tanh)
                    nc.scalar.activation(sall[:, 1536:2048], z[:, 1536:2048],
                                         AF.Sigmoid)
                    tg = sall[:, 0:512]
                    si = sall[:, 512:1024]
                    sf = sall[:, 1024:1536]
                    so = sall[:, 1536:2048]
                    cnew = work.tile([128, 512], BF16, tag=f"c{d}",
                                     name=f"c{d}_{t}")
                    if t == 0:
                        nc.vector.tensor_tensor(cnew[:], tg, si, ALU.mult)
                    else:
                        v = work.tile([128, 512], BF16, tag=f"v{d}",
                                      name=f"v{d}_{t}")
                        nc.vector.tensor_tensor(
                            v[:], sf, c_prev[d][:], ALU.mult)
                        m1 = work.tile([128, 512], BF16, tag=f"m1{d}",
                                       name=f"m1{d}_{t}")
                        nc.vector.tensor_tensor(m1[:], tg, si, ALU.mult)
                        nc.vector.tensor_tensor(cnew[:], m1[:], v[:], ALU.add)
                    c_prev[d] = cnew
                    th = work.tile([128, 512], BF16, tag=f"th{d}",
                                   name=f"th{d}_{t}")
                    nc.scalar.activation(th[:], cnew[:], AF.Tanh)
                    h = work.tile([128, 512], BF16, tag=f"h{d}",
                                  name=f"h{d}_{t}")
                    nc.vector.tensor_tensor(h[:], so, th[:], ALU.mult)
                    nc.vector.tensor_tensor(
                        hmax[d][e][:], hmax[d][e][:], h[:], ALU.max)
                    # lag-1 h delta for this quad's next step
                    if 1 <= t <= L - 2:
                        dnew = work.tile([128, 512], BF16, tag=f"dh{d}",
                                         bufs=2, name=f"dh{d}_{t}")
                        nc.vector.tensor_tensor(
                            dnew[:], h[:], h_prev[d][:], ALU.subtract)
                        dh[d] = dnew
                    h_prev[d] = h

            for d in range(NQ):
                for e in range(3):
                    off = (d * 3 + e) * 512
                    nc.sync.dma_start(out[:, off:off + 512], hmax[d][e][:])

    nc.compile()
    return nc


def _chain_meta():
    """Global chain table: (dir, seg_idx, aw) per (core, slot).

    slot = d*CQ + c4; segment j = 4*core + c4.
    """
    meta = []
    for core in range(NCORES):
        row = []
        for slot in range(NQ * CQ):
            d, c4 = slot // CQ, slot % CQ
            j = 4 * core + c4
            aw = 0 if j == 0 else STRIDE * j - W
            row.append((d, j, aw))
        meta.append(row)
    return meta


def _pack_blobs(X, weights):
    bf = ml_dtypes.bfloat16
    perm = np.concatenate(
        [np.arange(r * 128, (r + 1) * 128) for r in GATE_ROW_PERM]
    )

    def lhsT_img(Wm):
        img = np.empty((128, KT * GT * 128), np.float32)
        for k in range(KT):
            for t8 in range(GT):
                blockT = Wm[t8 * 128:(t8 + 1) * 128, k * 128:(k + 1) * 128].T
                img[:, (k * GT + t8) * 128:(k * GT + t8 + 1) * 128] = blockT
        return img

    wimg = np.zeros((128, WCOLS), np.float32)
    for d, nm in enumerate("fb"):
        wih_p = weights[f"wih_{nm}"][perm].copy()
        whh_p = weights[f"whh_{nm}"][perm].copy()
        bias_p = (weights[f"bih_{nm}"] + weights[f"bhh_{nm}"])[perm].copy()
        wimg[:, WIH_OFF + d * 2048:WIH_OFF + (d + 1) * 2048] = lhsT_img(wih_p)
        wimg[:, WHH_OFF + d * 2048:WHH_OFF + (d + 1) * 2048] = lhsT_img(whh_p)
        for bank in range(4):
            off = BM_OFF + (d * 4 + bank) * 128
            for j in range(2):
                wimg[j, off:off + 128] = bias_p[(bank * 2 + j) * 128:
                                                (bank * 2 + j + 1) * 128]
    for j in range(2):
        wimg[j, IND_OFF + j * 256:IND_OFF + (j + 1) * 256] = 1.0
    wimg = wimg.astype(bf)

    # X as [k, 128, tok, b]
    Xt = np.ascontiguousarray(
        np.transpose(X.reshape(S, B, KT, 128), (2, 3, 0, 1))
    )

    meta = _chain_meta()
    xblobs = []
    for core in range(NCORES):
        img = np.zeros((128, XCOLS), np.float32)
        for slot in range(NQ * CQ):
            d, j, aw = meta[core][slot]
            c4 = slot % CQ
            lo = aw // 2
            gid = [min(lo + RT[d](t), S - 1) for t in range(L)]
            # error-compensated deltas: track the bank's effective
            # accumulated token (fp32 sum of the emitted bf16 deltas)
            xeff = None
            for t in range(L):
                if not _has_wih(d, t):
                    continue
                tgt = Xt[:, :, gid[t], :]                    # (KT, 128, B)
                if xeff is None:
                    dxb = tgt.astype(bf).astype(np.float32)
                    xeff = dxb.copy()
                else:
                    dxb = (tgt - xeff).astype(bf).astype(np.float32)
                    xeff = xeff + dxb
                for k in range(KT):
                    col = t * XROW + k * 512 + d * 256 + c4 * 64
                    img[:, col:col + B] = dxb[k]
        xblobs.append(img.astype(bf))
    return wimg, xblobs


_PROGRAM_CACHE = {}


def _get_program():
    if "nc" not in _PROGRAM_CACHE:
        _PROGRAM_CACHE["nc"] = _build_program()
    return _PROGRAM_CACHE["nc"]


def _run(inputs, trace=False):
    X = np.asarray(inputs["inputs"], np.float32)
    wimg, xblobs = _pack_blobs(X, inputs)
    nc = _get_program()
    in_maps = [{"wblob": wimg, "xblob": xb} for xb in xblobs]
    res = run_bass_kernel_spmd(
        nc, in_maps, core_ids=list(range(NCORES)), trace=trace
    )
    meta = _chain_meta()
    emb = np.full((2, B, H), -np.inf, np.float32)
    for core in range(NCORES):
        o = np.asarray(res.results[core]["out"], np.float32)
        for slot in range(NQ * CQ):
            d, j, aw = meta[core][slot]
            c4 = slot % CQ
            epochs = [1]
            if j == 0:
                epochs.append(0)
            if aw + L - 1 < NT:
                epochs.append(2)
            for e in epochs:
                off = (d * 3 + e) * 512
                blk = o[:, off:off + 512].reshape(128, 2, 4, 64)
                cur = blk[:, :, c4, :]             # (p, X, b)
                cur = np.transpose(cur, (2, 1, 0)).reshape(B, H)
                emb[d] = np.maximum(emb[d], cur)
    return np.concatenate([emb[0], emb[1]], axis=-1), res


def kernel(**inputs):
    emb, _ = _run(inputs, trace=False)
    return emb
